# revision 8
# baseline (speedup 1.0000x reference)
"""BarCachedCrossAttention Trainium2 kernel.

Sharding: 8 cores = 4 batches x 2 head-groups (8 heads / 512 channels each).
Per core, everything is computed in a transposed layout (partition = context
token for scores) so that:
  - the instrument mask + 1/sqrt(d) scale fold into the single ACT exp op
    (mask depends only on the partition index there),
  - probs never need a transpose: U^T = V'^T @ P^T with a ones-column in V
    producing the softmax denominators for free,
  - the embedding gather becomes one K=128 matmul against a 128-row
    (instrument x bar) combined table.
K-bias is dropped (exactly cancels in softmax over n); Q-bias + current
instrument embedding fold into a host-prepped per-channel bias; V-bias rides
the combo table via a ones-row matmul.
"""

import sys

sys.path.insert(0, "/opt/trn_rl_repo")

import numpy as np

import concourse.bacc as bacc
import concourse.tile as tile
from concourse import mybir
from concourse.bass_utils import run_bass_kernel_spmd

B, T, N_CTX, H = 4, 512, 2048, 1024
NUM_HEADS, NUM_INSTRUMENTS, MAX_BARS = 16, 16, 8
HEAD_DIM = H // NUM_HEADS  # 64
HG = 2  # head groups (cores per batch)
CH = H // HG  # 512 channels per core
NH_G = NUM_HEADS // HG  # 8 heads per core
COMBO = NUM_INSTRUMENTS * MAX_BARS  # 128
P = 128
F32 = mybir.dt.float32
NEG = -30000.0  # additive mask; exp(NEG + s/8) underflows to exact 0

KC = H // P  # 8 contraction chunks for projections
PT_CH = CH // P  # 4 partition tiles of channels
NS = N_CTX // 512  # 4 context slabs of 512 tokens
NT = N_CTX // P  # 16 context tiles of 128 tokens
TT = T // P  # 4 tiles of query tokens

_compiled = None


def _build():
    nc = bacc.Bacc("TRN2", target_bir_lowering=False, debug=False, num_devices=8)

    qT_d = nc.dram_tensor("qT", [H, T], F32, kind="ExternalInput")
    ctxT_d = nc.dram_tensor("ctxT", [H, N_CTX], F32, kind="ExternalInput")
    ct_d = nc.dram_tensor("ct", [H, COMBO], F32, kind="ExternalInput")
    oh_d = nc.dram_tensor("oh", [COMBO, N_CTX], F32, kind="ExternalInput")
    wq_d = nc.dram_tensor("wqT", [H, CH], F32, kind="ExternalInput")
    wk_d = nc.dram_tensor("wkT", [H, CH], F32, kind="ExternalInput")
    wv_d = nc.dram_tensor("wvT", [H, CH], F32, kind="ExternalInput")
    wo_d = nc.dram_tensor("woT", [CH, H], F32, kind="ExternalInput")
    mb_d = nc.dram_tensor("mb", [P, NT], F32, kind="ExternalInput")
    bqe_d = nc.dram_tensor("bqe", [P, PT_CH], F32, kind="ExternalInput")
    bvg_d = nc.dram_tensor("bvg", [1, CH], F32, kind="ExternalInput")
    out_d = nc.dram_tensor("out", [T, H], F32, kind="ExternalOutput")

    with tile.TileContext(nc) as tc:
        with tc.tile_pool(name="persist", bufs=1) as pers:
            mb = pers.tile([P, NT], F32, name="mb")
            nc.sync.dma_start(mb[:], mb_d.ap())
            bqe = pers.tile([P, PT_CH], F32, name="bqe")
            nc.sync.dma_start(bqe[:], bqe_d.ap())
            bvg = pers.tile([1, CH], F32, name="bvg")
            nc.sync.dma_start(bvg[:], bvg_d.ap())
            ones1 = pers.tile([1, P], F32, name="ones1")
            nc.vector.memset(ones1[:], 1.0)
            ct = pers.tile([P, KC, COMBO], F32, name="ct")
            for k in range(KC):
                nc.sync.dma_start(ct[:, k, :], ct_d.ap()[k * P : (k + 1) * P, :])
            oh = pers.tile([P, N_CTX], F32, name="oh")
            nc.sync.dma_start(oh[:], oh_d.ap())

            KT = [pers.tile([P, N_CTX], F32, name=f"kt{p}") for p in range(PT_CH)]
            QT = [pers.tile([P, T], F32, name=f"qt{p}") for p in range(PT_CH)]
            OT = [pers.tile([P, T], F32, name=f"ot{p}") for p in range(PT_CH)]
            Vt = [pers.tile([P, NH_G, HEAD_DIM + 1], F32, name=f"v{i}") for i in range(NT)]
            ck = pers.tile([P, CH], F32, name="ck")
            cv = pers.tile([P, CH], F32, name="cv")

            # ---- Q projection: QT[p] = (Wq_g @ q^T)[p] + bq_eff ----
            with (
                tc.tile_pool(name="qw", bufs=1) as qwp,
                tc.tile_pool(name="qps", bufs=2, space="PSUM") as qps,
            ):
                qt = qwp.tile([P, KC, T], F32, name="qt_in")
                wq = qwp.tile([P, KC, CH], F32, name="wq")
                for k in range(KC):
                    nc.sync.dma_start(qt[:, k, :], qT_d.ap()[k * P : (k + 1) * P, :])
                    nc.sync.dma_start(wq[:, k, :], wq_d.ap()[k * P : (k + 1) * P, :])
                for p in range(PT_CH):
                    ps = qps.tile([P, 512], F32, name="ps_q")
                    for k in range(KC):
                        nc.tensor.matmul(
                            ps[:],
                            wq[:, k, p * P : (p + 1) * P],
                            qt[:, k, :],
                            start=(k == 0),
                            stop=(k == KC - 1),
                        )
                    nc.scalar.activation(
                        QT[p][:], ps[:], mybir.ActivationFunctionType.Identity,
                        bias=bqe[:, p : p + 1], scale=1.0,
                    )

            # ---- K^T / V projections + combo-table embedding adds ----
            with (
                tc.tile_pool(name="kv", bufs=1) as kvp,
                tc.tile_pool(name="kvps", bufs=2, space="PSUM") as kvps,
            ):
                wk = kvp.tile([P, KC, CH], F32, name="wk")
                wv = kvp.tile([P, KC, CH], F32, name="wv")
                for k in range(KC):
                    nc.sync.dma_start(wk[:, k, :], wk_d.ap()[k * P : (k + 1) * P, :])
                    nc.sync.dma_start(wv[:, k, :], wv_d.ap()[k * P : (k + 1) * P, :])

                # combo tables CK = C @ Wk.T (no bias), CV = C @ Wv.T + bv
                ps_ck = kvps.tile([P, 512], F32, name="ps_ck", bufs=1)
                for k in range(KC):
                    nc.tensor.matmul(
                        ps_ck[:], ct[:, k, :], wk[:, k, :],
                        start=(k == 0), stop=(k == KC - 1),
                    )
                nc.vector.tensor_copy(ck[:], ps_ck[:])
                ps_cv = kvps.tile([P, 512], F32, name="ps_cv", bufs=1)
                for k in range(KC):
                    nc.tensor.matmul(
                        ps_cv[:], ct[:, k, :], wv[:, k, :],
                        start=(k == 0), stop=False,
                    )
                nc.tensor.matmul(ps_cv[:], ones1[:], bvg[:], start=False, stop=True)
                nc.vector.tensor_copy(cv[:], ps_cv[:])

                with tc.tile_pool(name="slab", bufs=2) as slabp:
                    for ns in range(NS):
                        slab = slabp.tile([P, KC, 512], F32, name="slab")
                        for k in range(KC):
                            nc.sync.dma_start(
                                slab[:, k, :],
                                ctxT_d.ap()[k * P : (k + 1) * P, ns * 512 : (ns + 1) * 512],
                            )
                        for p in range(PT_CH):
                            ps = kvps.tile([P, 512], F32, name="ps_kt")
                            for k in range(KC):
                                nc.tensor.matmul(
                                    ps[:],
                                    wk[:, k, p * P : (p + 1) * P],
                                    slab[:, k, :],
                                    start=(k == 0), stop=False,
                                )
                            nc.tensor.matmul(
                                ps[:],
                                ck[:, p * P : (p + 1) * P],
                                oh[:, ns * 512 : (ns + 1) * 512],
                                start=False, stop=True,
                            )
                            nc.vector.tensor_copy(KT[p][:, ns * 512 : (ns + 1) * 512], ps[:])
                        for s4 in range(4):
                            i = ns * 4 + s4
                            psv = kvps.tile([P, 512], F32, name="ps_v")
                            for k in range(KC):
                                nc.tensor.matmul(
                                    psv[:],
                                    slab[:, k, s4 * P : (s4 + 1) * P],
                                    wv[:, k, :],
                                    start=(k == 0), stop=False,
                                )
                            nc.tensor.matmul(
                                psv[:], oh[:, i * P : (i + 1) * P], cv[:],
                                start=False, stop=True,
                            )
                            nc.vector.tensor_copy(
                                Vt[i][:, :, :HEAD_DIM],
                                psv[:].rearrange("p (h d) -> p h d", d=HEAD_DIM),
                            )
                            nc.vector.memset(Vt[i][:, :, HEAD_DIM : HEAD_DIM + 1], 1.0)

            # ---- attention + output projection ----
            with tc.tile_pool(name="att", bufs=1) as attp:
                wo = attp.tile([P, PT_CH, H], F32, name="wo")
                for p in range(PT_CH):
                    nc.sync.dma_start(wo[:, p, :], wo_d.ap()[p * P : (p + 1) * P, :])

                with (
                    tc.tile_pool(name="ptp", bufs=6) as ptp,
                    tc.tile_pool(name="usb", bufs=2) as usb,
                    tc.tile_pool(name="sps", bufs=4, space="PSUM") as sps,
                    tc.tile_pool(name="ups", bufs=2, space="PSUM") as ups,
                    tc.tile_pool(name="rps", bufs=1, space="PSUM") as rps,
                ):
                    for h in range(NH_G):
                        p, hi = h // 2, h % 2
                        d0, d1 = hi * HEAD_DIM, (hi + 1) * HEAD_DIM
                        psu = ups.tile([HEAD_DIM + 1, 512], F32, name="ps_u")
                        for i in range(NT):
                            pss = sps.tile([P, 512], F32, name="ps_s")
                            nc.tensor.matmul(
                                pss[:],
                                KT[p][d0:d1, i * P : (i + 1) * P],
                                QT[p][d0:d1, :],
                                start=True, stop=True,
                            )
                            pt = ptp.tile([P, 512], F32, name="pt")
                            nc.scalar.activation(
                                pt[:], pss[:], mybir.ActivationFunctionType.Exp,
                                bias=mb[:, i : i + 1], scale=0.125,
                            )
                            nc.tensor.matmul(
                                psu[:], Vt[i][:, h, :], pt[:],
                                start=(i == 0), stop=(i == NT - 1),
                            )
                        u = usb.tile([HEAD_DIM + 1, 512], F32, name="u")
                        nc.vector.tensor_copy(u[:], psu[:])
                        r = usb.tile([1, 512], F32, name="r")
                        nc.vector.reciprocal(r[:], u[HEAD_DIM : HEAD_DIM + 1, :])
                        psr = rps.tile([HEAD_DIM, 512], F32, name="ps_r")
                        nc.tensor.matmul(psr[:], ones1[:, :HEAD_DIM], r[:], start=True, stop=True)
                        nc.vector.tensor_tensor(
                            OT[p][d0:d1, :], u[:HEAD_DIM, :], psr[:],
                            op=mybir.AluOpType.mult,
                        )

                # O = OT.T @ WoT (partial over this head-group's channels)
                with (
                    tc.tile_pool(name="ob", bufs=3) as obp,
                    tc.tile_pool(name="ops", bufs=2, space="PSUM") as ops,
                ):
                    for tt in range(TT):
                        for o in range(2):
                            pso = ops.tile([P, 512], F32, name="ps_o")
                            for p in range(PT_CH):
                                nc.tensor.matmul(
                                    pso[:],
                                    OT[p][:, tt * P : (tt + 1) * P],
                                    wo[:, p, o * 512 : (o + 1) * 512],
                                    start=(p == 0), stop=(p == PT_CH - 1),
                                )
                            ob = obp.tile([P, 512], F32, name="ob")
                            nc.vector.tensor_copy(ob[:], pso[:])
                            nc.sync.dma_start(
                                out_d.ap()[tt * P : (tt + 1) * P, o * 512 : (o + 1) * 512],
                                ob[:],
                            )

    nc.compile()
    return nc


def _prep_inputs(query, context, instrument_ids, current_instrument_id, bar_offsets,
                 Wq, bq, Wk, bk, Wv, bv, Wo, bo, inst_emb, bar_emb):
    f32 = np.float32
    query = np.asarray(query, f32)
    context = np.asarray(context, f32)
    inst = np.asarray(instrument_ids).astype(np.int64)
    bars = np.clip(np.asarray(bar_offsets).astype(np.int64), 0, MAX_BARS - 1)
    cur = int(np.asarray(current_instrument_id))
    Wq, Wk, Wv, Wo = (np.asarray(w, f32) for w in (Wq, Wk, Wv, Wo))
    bq, bv, bo = (np.asarray(b, f32) for b in (bq, bv, bo))
    inst_emb = np.asarray(inst_emb, f32)
    bar_emb = np.asarray(bar_emb, f32)

    C = (inst_emb[:, None, :] + bar_emb[None, :, :]).reshape(COMBO, H)
    ctT = np.ascontiguousarray(C.T)  # (H, 128)
    bq_eff = bq + inst_emb[cur] @ Wq.T  # (H,)
    WqT = np.ascontiguousarray(Wq.T)
    WkT = np.ascontiguousarray(Wk.T)
    WvT = np.ascontiguousarray(Wv.T)
    WoT = np.ascontiguousarray(Wo.T)

    combo = inst * MAX_BARS + bars  # (B, N)
    ar = np.arange(COMBO)[:, None]

    in_maps = []
    for b in range(B):
        qT = np.ascontiguousarray(query[b].T)
        ctxT = np.ascontiguousarray(context[b].T)
        ohT = (combo[b][None, :] == ar).astype(f32)  # (128, N)
        mbv = np.where(inst[b] == cur, NEG, 0.0).astype(f32)
        mbt = np.ascontiguousarray(mbv.reshape(NT, P).T)  # (128, NT)
        for g in range(HG):
            sl = slice(g * CH, (g + 1) * CH)
            in_maps.append({
                "qT": qT,
                "ctxT": ctxT,
                "ct": ctT,
                "oh": ohT,
                "wqT": np.ascontiguousarray(WqT[:, sl]),
                "wkT": np.ascontiguousarray(WkT[:, sl]),
                "wvT": np.ascontiguousarray(WvT[:, sl]),
                "woT": np.ascontiguousarray(WoT[sl, :]),
                "mb": mbt,
                "bqe": np.ascontiguousarray(bq_eff[sl].reshape(PT_CH, P).T),
                "bvg": bv[sl].reshape(1, CH),
            })
    return in_maps, bo


def kernel(**inputs) -> np.ndarray:
    global _compiled
    if _compiled is None:
        _compiled = _build()
    in_maps, bo = _prep_inputs(**inputs)
    res = run_bass_kernel_spmd(_compiled, in_maps, list(range(B * HG))).results
    out = np.empty((B, T, H), np.float32)
    for b in range(B):
        out[b] = res[b * HG]["out"] + res[b * HG + 1]["out"] + bo
    return out
